# revision 1
# baseline (speedup 1.0000x reference)
"""Chamfer distance loss kernel for Trainium2 (8 NeuronCores).

Strategy
--------
d(n, m) = ||x_n||^2 + ||y_m||^2 - 2 x_n . y_m  is produced directly by the
TensorEngine with a K=5 augmented contraction:
    lhsT rows = [x, y, z, xx, 1]          (predict side, [5, Np])
    rhs  rows = [-2tx, -2ty, -2tz, 1, yy] (target side,  [5, M])
so each matmul emits a [128, 512] tile of the full distance matrix into PSUM.

Sharding: batch b = core//2 on each pair of cores; each core takes half of the
predict rows (4096) and the full 8192-point target set.  Per core:
  * x-direction: row-min over the free dim (min over all targets for each of
    its 4096 predict rows), via a TT-min tree + reduce on the VectorE.
  * y-direction: running elementwise min across row-chunks (col-min partials,
    min over this core's 4096 predict rows), finished on the host with a
    partition-min and a cross-core (pair) min.

The PSUM tiles are evacuated by the ScalarE as bf16.  bf16 rounding is
monotonic, so min over rounded values == rounded true min; the final scalar
only sees ~1e-5 relative error from this.  All elementwise mins run as int16
mins on the bf16 bit patterns (order-isomorphic for non-negative values —
true for these distances, whose fp32 values stay well above 0): the int16
TensorTensor path runs ~8x faster than the float-min ALU path on TRN2.

The matmul inputs are 3-way bf16 splits (hi/mid/lo) of the fp32 operands;
keeping all cross terms down to O(2^-27) reproduces fp32-accurate distances
while streaming at the PE's full bf16 rate (fp32 matmul is 4x slower).
"""

import sys

sys.path.insert(0, "/opt/trn_rl_repo")

import numpy as np

B = 4
N = 8192  # predict points per batch
M = 8192  # target points per batch
NCORES = 8
HALF = N // 2  # predict rows per core (2 cores per batch)

ROW_CHUNKS = HALF // 128  # 32 chunks of 128 predict rows
COL_STRIPS = M // 512  # 16 strips of 512 target cols
STRIP_GROUPS = COL_STRIPS // 4  # 4 groups of 4 strips (one 4-bank PSUM tile)

_CACHE = {}


K_AUG = 24  # 3-way bf16 split: 18 coord rows + 3 xx rows + 3 yy rows


def _build_nc(repeats=1, hw_loop=1, acc_bf16=True, gps8=0, colmin_dma=False,
              evac_dve8=0, no_colmin=False, wide_sb=True, fake_add=False,
              relu_evac=False, i16_min=True,
              no_rowmin=False, sbc_bufs=2, psum_bufs=2, act_evac=True):
    """Build the SPMD single-core Bass program (same program on all 8 cores).

    repeats: run the main loop this many times (idempotent — used for timing).
    gpsimd_jgs: strip-group indices whose col-min chain runs on GPSIMD.
    """
    import concourse.bass as bass  # noqa: F401
    import concourse.mybir as mybir
    import concourse.tile as tile
    from concourse import bacc

    f32 = mybir.dt.float32
    bf16 = mybir.dt.bfloat16
    i16 = mybir.dt.int16
    acc_dt = bf16 if acc_bf16 else f32
    AluOp = mybir.AluOpType
    MINOP = AluOp.add if fake_add else AluOp.min

    def ttmin(out, a, b):
        # min on non-negative bf16 == int16 min on the bit patterns, and the
        # int16 TT path runs at the fast packed rate while the bf16-min ALU
        # path measures ~8x slower.  Distances here are strictly positive
        # (min true distance ~1e-3 >> fp32 rounding), so this is exact.
        if fake_add:
            nc.vector.tensor_tensor(out, a, b, op=AluOp.add)
        elif i16_min:
            nc.vector.tensor_tensor(out.bitcast(i16), a.bitcast(i16),
                                    b.bitcast(i16), op=AluOp.min)
        else:
            nc.vector.tensor_tensor(out, a, b, op=AluOp.min)

    nc = bacc.Bacc("TRN2", target_bir_lowering=False, debug=False, num_devices=NCORES)
    lhs_d = nc.dram_tensor("lhs", [K_AUG, HALF], bf16, kind="ExternalInput")
    rhs_d = nc.dram_tensor("rhs", [K_AUG, M], bf16, kind="ExternalInput")
    xm_d = nc.dram_tensor("xm", [128, ROW_CHUNKS], f32, kind="ExternalOutput")
    ym_d = nc.dram_tensor("ym", [128, M], acc_dt, kind="ExternalOutput")

    with tile.TileContext(nc) as tc:
        with (
            tc.tile_pool(name="persist", bufs=1) as persist,
            tc.tile_pool(name="sbc", bufs=sbc_bufs) as sbc,
            tc.tile_pool(name="tru", bufs=2) as tru,
            tc.tile_pool(name="trv", bufs=2) as trv,
            tc.tile_pool(name="tr1", bufs=2) as tr1,
            tc.tile_pool(name="tr2", bufs=2) as tr2,
            tc.tile_pool(name="tr3", bufs=2) as tr3,
            tc.tile_pool(name="psum", bufs=psum_bufs, space="PSUM") as psum,
        ):
            lhs = persist.tile([K_AUG, HALF], bf16)
            rhs = persist.tile([K_AUG, M], bf16)
            acc = persist.tile([128, M], acc_dt)
            rowp = persist.tile([128, ROW_CHUNKS], f32)
            nc.gpsimd.dma_start(lhs[:], lhs_d[:])
            nc.gpsimd.dma_start(rhs[:], rhs_d[:])

            import contextlib

            loop_cm = (tc.For_i(0, hw_loop, 1) if hw_loop > 1
                       else contextlib.nullcontext())
            with loop_cm:
              for rep in range(repeats):
                for i in range(ROW_CHUNKS):
                    sb = sbc.tile([128, M], acc_dt)
                    for jg in range(STRIP_GROUPS):
                        pt = psum.tile([128, 2048], f32)
                        for k in range(4):
                            j = jg * 4 + k
                            nc.tensor.matmul(
                                pt[:, k * 512:(k + 1) * 512],
                                lhs[:, i * 128:(i + 1) * 128],
                                rhs[:, j * 512:(j + 1) * 512],
                                start=True,
                                stop=True,
                            )
                        # Evacuate PSUM -> SBUF (bf16 cast) on ScalarE.
                        sl = sb[:, jg * 2048:(jg + 1) * 2048]
                        if ((i * STRIP_GROUPS + jg) % 8) < evac_dve8:
                            nc.vector.tensor_copy(sl, pt[:])
                        elif relu_evac:
                            # ReLU clamps fp32-rounding negatives so the
                            # int16-min trick is exact.
                            nc.scalar.activation(
                                sl, pt[:],
                                mybir.ActivationFunctionType.Relu)
                        else:
                            nc.scalar.copy(sl, pt[:])
                    # Col-min running accumulate (one wide TT, 2x on bf16).
                    if i == 0 and rep == 0:
                        nc.vector.tensor_copy(acc[:], sb[:])
                    elif not no_colmin:
                        ttmin(acc[:], sb[:], acc[:])
                    # Row-min: TT-min halving tree + final reduce.
                    if no_rowmin and i > 0:
                        continue
                    t0 = tru.tile([128, 4096], acc_dt)
                    ttmin(t0[:], sb[:, :4096], sb[:, 4096:])
                    t1 = tr1.tile([128, 2048], acc_dt)
                    ttmin(t1[:], t0[:, :2048], t0[:, 2048:])
                    t2 = tr2.tile([128, 1024], acc_dt)
                    ttmin(t2[:], t1[:, :1024], t1[:, 1024:])
                    t3 = tr3.tile([128, 512], acc_dt)
                    ttmin(t3[:], t2[:, :512], t2[:, 512:])
                    t4 = trv.tile([128, 256], acc_dt)
                    ttmin(t4[:], t3[:, :256], t3[:, 256:])
                    nc.vector.tensor_reduce(
                        out=rowp[:, i:i + 1], in_=t4[:],
                        axis=mybir.AxisListType.X, op=AluOp.min,
                    )

            nc.gpsimd.dma_start(xm_d[:], rowp[:])
            nc.gpsimd.dma_start(ym_d[:], acc[:])

    nc.compile()
    return nc


def _get_nc(**kw):
    key = tuple(sorted(kw.items()))
    if key not in _CACHE:
        _CACHE[key] = _build_nc(**kw)
    return _CACHE[key]


def _split3(x):
    """fp32 -> (hi, mid, lo) bf16 triplet with hi+mid+lo ~ x to ~2^-25."""
    import ml_dtypes

    bf = ml_dtypes.bfloat16
    h = x.astype(bf)
    r = x - h.astype(np.float32)
    m = r.astype(bf)
    r2 = r - m.astype(np.float32)
    l = r2.astype(bf)
    return h, m, l


def _prep_in_maps(predict, target):
    """Host-side shard + transpose + augment (tiny: ~3MB total).

    d = sum_k lhs[k,n] * rhs[k,m] reproduces xx + yy - 2 x.y to fp32-level
    accuracy using 3-way bf16 splits: for each scalar product a*b with
    a=ah+am+al, b=bh+bm+bl we keep ah*(bh+bm+bl) + am*(bh+bm) + al*bh;
    dropped terms are O(2^-27).
    """
    import ml_dtypes

    bf = ml_dtypes.bfloat16
    predict = np.asarray(predict, dtype=np.float32)
    target = np.asarray(target, dtype=np.float32)
    in_maps = []
    for c in range(NCORES):
        b, h = divmod(c, 2)
        p = predict[b, h * HALF:(h + 1) * HALF, :]  # [HALF, 3]
        t = target[b]  # [M, 3]
        xx = (p * p).sum(axis=1)
        yy = (t * t).sum(axis=1)
        ph, pm, pl = _split3(p.T)            # [3, HALF] each
        th, tm, tl = _split3(-2.0 * t.T)     # [3, M] each
        xh, xm, xl = _split3(xx[None, :])    # [1, HALF]
        yh, ym, yl = _split3(yy[None, :])    # [1, M]
        one = np.ones((1,), dtype=bf)
        lhs = np.empty((K_AUG, HALF), dtype=bf)
        rhs = np.empty((K_AUG, M), dtype=bf)
        r = 0
        for cd in range(3):  # coordinate products
            # ah*bh, ah*bm, ah*bl, am*bh, am*bm, al*bh
            for a, bb in ((ph, th), (ph, tm), (ph, tl),
                          (pm, th), (pm, tm), (pl, th)):
                lhs[r] = a[cd]
                rhs[r] = bb[cd]
                r += 1
        for a in (xh, xm, xl):  # xx * 1
            lhs[r] = a[0]
            rhs[r] = one
            r += 1
        for bb in (yh, ym, yl):  # 1 * yy
            lhs[r] = one
            rhs[r] = bb[0]
            r += 1
        assert r == K_AUG
        in_maps.append({"lhs": np.ascontiguousarray(lhs),
                        "rhs": np.ascontiguousarray(rhs)})
    return in_maps


def _postprocess(results):
    """Combine per-core partials into the scalar loss."""
    xsum = 0.0
    ysum = 0.0
    for b in range(B):
        r0 = results[2 * b]
        r1 = results[2 * b + 1]
        xsum += float(r0["xm"].astype(np.float64).sum())
        xsum += float(r1["xm"].astype(np.float64).sum())
        ym0 = r0["ym"].astype(np.float32).min(axis=0)
        ym1 = r1["ym"].astype(np.float32).min(axis=0)
        ysum += np.minimum(ym0, ym1).astype(np.float64).sum()
    total = xsum / (B * N) + ysum / (B * M)
    return np.float32(total)


def _run(in_maps, **build_kw):
    from concourse.bass_utils import run_bass_kernel_spmd

    nc = _get_nc(**build_kw)
    res = run_bass_kernel_spmd(nc, in_maps, core_ids=list(range(NCORES)))
    return res.results


def kernel(predict, target):
    in_maps = _prep_in_maps(predict, target)
    results = _run(in_maps)
    return _postprocess(results)


if __name__ == "__main__":
    rng = np.random.default_rng(0)
    predict = rng.standard_normal((B, N, 3)).astype(np.float32)
    target = rng.standard_normal((B, M, 3)).astype(np.float32)
    out = kernel(predict, target)
    # numpy reference
    exp_x = 0.0
    exp_y = 0.0
    for b in range(B):
        d = ((predict[b][:, None, :] - target[b][None, :, :]) ** 2).sum(-1)
        exp_x += d.min(axis=1).sum()
        exp_y += d.min(axis=0).sum()
    exp = exp_x / (B * N) + exp_y / (B * M)
    print("kernel:", out, "expected:", exp, "rel err:",
          abs(out - exp) / abs(exp))



# revision 11
# speedup vs baseline: 11.7672x; 11.7672x over previous
"""Chamfer distance loss kernel for Trainium2 (8 NeuronCores) — norm-banded.

Strategy
--------
d(n, m) = ||x_n||^2 + ||y_m||^2 - 2 x_n . y_m  is produced by the TensorEngine
with a K=24 augmented contraction (3-way bf16 splits of the fp32 operands keep
fp32-level accuracy at the PE's bf16 streaming rate).

Band pruning: both point sets are sorted by norm on the host.  Since
d(x, y) >= (||x|| - ||y||)^2, a point's nearest neighbour is norm-local: for
this input regime ~98% of NNs lie within +-640 sorted ranks.  Each 128-row
predict chunk therefore only computes distances to a window of targets around
its own rank position (per-chunk window widths M_TILDE, tuned with a safety
pad), i.e. ~14% of the full matrix.  Exactness is certified per row/column on
the host: computed_min <= (norm gap to the nearest excluded rank)^2 implies no
excluded target can beat it.  The rare rows/cols that fail the certificate
(~1-2%, norm-tail points) are recomputed exactly on the host.

Sharding: batch b = core//2; cores of a pair take interleaved global chunks
(parity h = core%2) to balance the variable window widths.  SPMD-compatible:
the program's window offsets assume parity 0; parity-1 cores receive an
rhs/acc layout shifted by 128 ranks so the same offsets select their windows.

Per chunk: PE matmuls -> PSUM [128, W]; ScalarE evacuates as bf16 (ReLU clamps
fp32-rounding negatives); VectorE does the whole row-min in ONE
tensor_tensor_reduce (pairwise min + fused min-reduce) and the running col-min
as an int16 TensorTensor on the bf16 bit patterns (order-isomorphic for
non-negative values; 2x DVE mode).  Col-min partials stream to DRAM behind the
sliding band; the host finishes with partition/parity mins + certificates.
"""

import sys

sys.path.insert(0, "/opt/trn_rl_repo")

import numpy as np

B = 4
N = 8192  # predict points per batch
M = 8192  # target points per batch
NCORES = 8
CH = 128          # predict rows per chunk
NCH = N // CH     # 64 global chunks per batch
LCH = NCH // 2    # 32 local chunks per core (interleaved by core parity)

# Per-local-chunk half-margins (in sorted target ranks).  m~_c covers both
# parities' chunks (max of the two global chunks it serves).  Tuned on the
# target input regime (q95 of needed margins + 64 pad); the host certificate
# + fallback keeps the kernel exact for ANY input regardless of this profile.
_M_RAW = [192, 192, 192, 256, 256, 320, 320, 384, 384, 384, 384, 448, 448,
          512, 448, 512, 448, 512, 576, 512, 576, 512, 576, 576, 576, 512,
          576, 576, 640, 576, 640, 576, 640, 640, 640, 640, 640, 640, 640,
          640, 640, 640, 640, 640, 640, 640, 576, 640, 640, 640, 576, 576,
          576, 576, 512, 512, 512, 448, 448, 448, 384, 320, 320, 256]
M_TILDE = [max(_M_RAW[2 * c], _M_RAW[2 * c + 1]) for c in range(LCH)]
W_LIST = [2 * m + 2 * CH for m in M_TILDE]  # window widths (256-row span + margins)
WMAX = max(W_LIST)

PAD = 768                      # sentinel pad; >= max(M_TILDE) + 128 parity shift
RHS_W = 2 * PAD + M            # rhs / acc local width
# window start (local cols) for local chunk c, parity-0 frame:
J0 = [256 * c - M_TILDE[c] + PAD for c in range(LCH)]
SENT = 100.0                   # sentinel coordinate -> distance ~3e4, never wins

K_AUG = 24  # 3-way bf16 split: 18 coord rows + 3 xx rows + 3 yy rows

_CACHE = {}


def _build_nc(repeats=1, hw_loop=1):
    """Build the SPMD single-core Bass program (same program on all 8 cores)."""
    import concourse.bass as bass  # noqa: F401
    import concourse.mybir as mybir
    import concourse.tile as tile
    from concourse import bacc

    f32 = mybir.dt.float32
    bf16 = mybir.dt.bfloat16
    i16 = mybir.dt.int16
    AluOp = mybir.AluOpType

    nc = bacc.Bacc("TRN2", target_bir_lowering=False, debug=False, num_devices=NCORES)
    lhs_d = nc.dram_tensor("lhs", [K_AUG, LCH * CH], bf16, kind="ExternalInput")
    rhs_d = nc.dram_tensor("rhs", [K_AUG, RHS_W], bf16, kind="ExternalInput")
    xm_d = nc.dram_tensor("xm", [128, LCH], bf16, kind="ExternalOutput")
    ym_d = nc.dram_tensor("ym", [128, RHS_W], bf16, kind="ExternalOutput")

    with tile.TileContext(nc) as tc:
        with (
            tc.tile_pool(name="persist", bufs=1) as persist,
            tc.tile_pool(name="sbp", bufs=3) as sbp,
            tc.tile_pool(name="t1p", bufs=2) as t1p,
            tc.tile_pool(name="t2p", bufs=2) as t2p,
            tc.tile_pool(name="t3p", bufs=2) as t3p,
            tc.tile_pool(name="psum", bufs=2, space="PSUM") as psum,
        ):
            lhs = persist.tile([K_AUG, LCH * CH], bf16)
            rhs = persist.tile([K_AUG, RHS_W], bf16)
            acc = persist.tile([128, RHS_W], bf16)
            rowp = persist.tile([128, LCH], bf16)
            # Piecewise input DMAs (HWDGE: no Q7 descriptor-gen serialization)
            # so early chunks unblock quickly.
            nc.sync.dma_start(rhs[:, :2048], rhs_d[:, :2048])
            nc.sync.dma_start(lhs[:, :1024], lhs_d[:, :1024])
            nc.sync.dma_start(rhs[:, 2048:5120], rhs_d[:, 2048:5120])
            nc.sync.dma_start(lhs[:, 1024:], lhs_d[:, 1024:])
            nc.sync.dma_start(rhs[:, 5120:], rhs_d[:, 5120:])

            import contextlib

            loop_cm = (tc.For_i(0, hw_loop, 1) if hw_loop > 1
                       else contextlib.nullcontext())
            with loop_cm:
              for rep in range(repeats):
                emitted = J0[0]
                covered = J0[0]  # acc cols [J0[0], covered) hold valid mins
                for c in range(LCH):
                    w = W_LIST[c]
                    j0 = J0[c]
                    pt = psum.tile([128, WMAX], f32)
                    off = 0
                    while off < w:
                        p = min(512, w - off)
                        nc.tensor.matmul(
                            pt[:, off:off + p],
                            lhs[:, c * CH:(c + 1) * CH],
                            rhs[:, j0 + off:j0 + off + p],
                            start=True,
                            stop=True,
                        )
                        off += p
                    sb = sbp.tile([128, WMAX], bf16)
                    # ReLU clamps fp32-rounding negatives so the int16-min
                    # trick stays exact.
                    nc.scalar.activation(sb[:, :w], pt[:, :w],
                                         mybir.ActivationFunctionType.Relu)
                    # Row-min: int16 TT-min halving tree (2x mode) + reduce.
                    def ttmin(out, a_, b_):
                        nc.vector.tensor_tensor(out.bitcast(i16), a_.bitcast(i16),
                                                b_.bitcast(i16), op=AluOp.min)
                    t1 = t1p.tile([128, WMAX // 2], bf16)
                    ttmin(t1[:, :w // 2], sb[:, :w // 2], sb[:, w // 2:w])
                    t2 = t2p.tile([128, WMAX // 4], bf16)
                    ttmin(t2[:, :w // 4], t1[:, :w // 4], t1[:, w // 4:w // 2])
                    t3 = t3p.tile([128, WMAX // 8], bf16)
                    ttmin(t3[:, :w // 8], t2[:, :w // 8], t2[:, w // 8:w // 4])
                    nc.vector.tensor_reduce(
                        out=rowp.bitcast(i16)[:, c:c + 1],
                        in_=t3.bitcast(i16)[:, :w // 8],
                        axis=mybir.AxisListType.X, op=AluOp.min)
                    # Running col-min (int16 on bf16 bit patterns: 2x mode).
                    # Cols entering the band for the first time are copied
                    # (4x mode) instead of min-folded — no acc init needed,
                    # and the copy keeps hw_loop iterations idempotent.
                    fold_hi = min(covered, j0 + w)
                    if fold_hi > j0:
                        accsl = acc[:, j0:fold_hi]
                        nc.vector.tensor_tensor(
                            accsl.bitcast(i16), sb.bitcast(i16)[:, :fold_hi - j0],
                            accsl.bitcast(i16), op=AluOp.min)
                    if j0 + w > covered:
                        nc.vector.tensor_copy(acc[:, covered:j0 + w],
                                              sb[:, covered - j0:w])
                        covered = j0 + w
                    # Stream out finalized col-min slices behind the band.
                    if c % 2 == 1:
                        hi = J0[c + 1] if c + 1 < LCH else J0[c] + w
                        if hi > emitted:
                            nc.sync.dma_start(ym_d[:, emitted:hi],
                                              acc[:, emitted:hi])
                            emitted = hi
                # Tail: remaining accumulator cols + row partials.
                tail_hi = J0[LCH - 1] + W_LIST[LCH - 1]
                if tail_hi > emitted:
                    nc.sync.dma_start(ym_d[:, emitted:tail_hi],
                                      acc[:, emitted:tail_hi])
                nc.sync.dma_start(xm_d[:], rowp[:])

    nc.compile()
    return nc


def _get_nc(**kw):
    key = tuple(sorted(kw.items()))
    if key not in _CACHE:
        _CACHE[key] = _build_nc(**kw)
    return _CACHE[key]


def _split3(x):
    """fp32 -> (hi, mid, lo) bf16 triplet with hi+mid+lo ~ x to ~2^-25."""
    import ml_dtypes

    bf = ml_dtypes.bfloat16
    h = x.astype(bf)
    r = x - h.astype(np.float32)
    m = r.astype(bf)
    r2 = r - m.astype(np.float32)
    l = r2.astype(bf)
    return h, m, l


def _sorted_arrays(predict, target):
    """Per-batch norm-sorted copies of both point sets."""
    out = []
    for b in range(B):
        p = np.asarray(predict[b], dtype=np.float32)
        t = np.asarray(target[b], dtype=np.float32)
        pn = np.linalg.norm(p, axis=1)
        tn = np.linalg.norm(t, axis=1)
        po = np.argsort(pn, kind="stable")
        to = np.argsort(tn, kind="stable")
        out.append((p[po], t[to], pn[po], tn[to]))
    return out


def _aug_pair(p, t):
    """Build the K=24 split-augmented (lhs_cols, rhs_cols) fp32->bf16 factors.

    p: [n, 3] predict-side points (lhs), t: [m, 3] target-side points (rhs).
    Returns lhs [24, n], rhs [24, m] such that sum_k lhs[k,i]*rhs[k,j]
    reproduces ||p_i - t_j||^2 to fp32-level accuracy.
    """
    import ml_dtypes

    bf = ml_dtypes.bfloat16
    xx = (p * p).sum(axis=1)
    yy = (t * t).sum(axis=1)
    ph, pm, pl = _split3(p.T)            # [3, n]
    th, tm, tl = _split3(-2.0 * t.T)     # [3, m]
    xh, xm_, xl = _split3(xx[None, :])
    yh, ym_, yl = _split3(yy[None, :])
    one_n = np.ones(p.shape[0], dtype=bf)
    one_m = np.ones(t.shape[0], dtype=bf)
    lhs = np.empty((K_AUG, p.shape[0]), dtype=bf)
    rhs = np.empty((K_AUG, t.shape[0]), dtype=bf)
    r = 0
    for cd in range(3):
        for a, bb in ((ph, th), (ph, tm), (ph, tl),
                      (pm, th), (pm, tm), (pl, th)):
            lhs[r] = a[cd]
            rhs[r] = bb[cd]
            r += 1
    for a in (xh, xm_, xl):
        lhs[r] = a[0]
        rhs[r] = one_m
        r += 1
    for bb in (yh, ym_, yl):
        lhs[r] = one_n
        rhs[r] = bb[0]
        r += 1
    assert r == K_AUG
    return lhs, rhs


def _prep_in_maps(predict, target):
    """Host-side sort + shard + augment (tiny: a few MB)."""
    sorted_arrs = _sorted_arrays(predict, target)
    in_maps = []
    for core in range(NCORES):
        b, h = divmod(core, 2)
        ps, ts, _, _ = sorted_arrs[b]
        #

        # This core's predict rows: global chunks 2c+h, c = 0..LCH-1.
        rows = np.concatenate(
            [np.arange(CH * (2 * c + h), CH * (2 * c + h + 1)) for c in range(LCH)])
        p_core = ps[rows]  # [LCH*CH, 3]
        # rhs layout: local col j <-> global target rank g = j - PAD + 128*h.
        g = np.arange(RHS_W) - PAD + CH * h
        t_loc = np.full((RHS_W, 3), SENT, dtype=np.float32)
        valid = (g >= 0) & (g < M)
        t_loc[valid] = ts[g[valid]]
        lhs, rhs = _aug_pair(p_core, t_loc)
        in_maps.append({"lhs": np.ascontiguousarray(lhs),
                        "rhs": np.ascontiguousarray(rhs)})
    return in_maps


def _run(in_maps, **build_kw):
    from concourse.bass_utils import run_bass_kernel_spmd

    nc = _get_nc(**build_kw)
    res = run_bass_kernel_spmd(nc, in_maps, core_ids=list(range(NCORES)))
    return res.results


def _postprocess(results, predict, target):
    """Stitch per-core partials; certify band exactness; fallback; sum."""
    sorted_arrs = _sorted_arrays(predict, target)
    SLACK = 0.98  # certificate slack for bf16 quantization of device mins
    xsum = 0.0
    ysum = 0.0
    for b in range(B):
        ps, ts, pn, tn = sorted_arrs[b]
        # ---- row direction (min over targets for each predict) ----
        rowm = np.empty(N, dtype=np.float64)
        for h in range(2):
            xm = results[2 * b + h]["xm"].astype(np.float64)  # [128, LCH]
            for c in range(LCH):
                i = 2 * c + h
                rowm[CH * i:CH * (i + 1)] = xm[:, c]
        # certificates
        rho = np.arange(N)
        c_of = (rho // CH) // 2
        mt = np.array(M_TILDE)[c_of]
        h_of = (rho // CH) % 2
        w_lo = 256 * c_of - mt + CH * h_of
        w_hi = w_lo + 2 * mt + 2 * CH
        lb = np.full(N, np.inf)
        has_lo = w_lo > 0
        lb[has_lo] = np.maximum(
            0.0, pn[rho[has_lo]] - tn[np.minimum(w_lo[has_lo] - 1, M - 1)]) ** 2
        has_hi = w_hi < M
        lb_hi = np.maximum(0.0, tn[w_hi[has_hi]] - pn[rho[has_hi]]) ** 2
        lb[has_hi] = np.minimum(lb[has_hi], lb_hi)
        bad = rowm > lb * SLACK
        for r in np.nonzero(bad)[0]:
            d = ((ps[r][None, :] - ts) ** 2).sum(axis=1)
            rowm[r] = float(d.min())
        xsum += rowm.sum()
        # ---- col direction (min over predicts for each target) ----
        colm = np.full(M, np.inf)
        gg = np.arange(M)
        for h in range(2):
            ym = results[2 * b + h]["ym"].astype(np.float32)  # [128, RHS_W]
            colpart = ym.min(axis=0).astype(np.float64)
            j = gg + PAD - CH * h
            ok = (j >= 0) & (j < RHS_W)
            colm[ok] = np.minimum(colm[ok], colpart[j[ok]])
        # Exact block-level coverage certificate: global chunk i covers
        # predict rows [128i, 128i+128) and target window [a_i, b_i).
        a_i = np.array([256 * (i // 2) - M_TILDE[i // 2] + CH * (i % 2)
                        for i in range(NCH)])
        b_i = a_i + np.array([2 * M_TILDE[i // 2] + 2 * CH for i in range(NCH)])
        covered = (gg[:, None] >= a_i[None, :]) & (gg[:, None] < b_i[None, :])
        blk_lo = pn[::CH]                       # [NCH] first norm of each block
        blk_hi = pn[CH - 1::CH]                 # [NCH] last norm of each block
        gap = np.maximum(blk_lo[None, :] - tn[:, None],
                         tn[:, None] - blk_hi[None, :])
        gap = np.maximum(gap, 0.0) ** 2         # [M, NCH] distance lb per block
        gap[covered] = np.inf
        lbc = gap.min(axis=1)
        badc = colm > lbc * SLACK
        for g in np.nonzero(badc)[0]:
            d = ((ps - ts[g][None, :]) ** 2).sum(axis=1)
            colm[g] = float(d.min())
        ysum += colm.sum()
    total = xsum / (B * N) + ysum / (B * M)
    return np.float32(total)


def kernel(predict, target):
    in_maps = _prep_in_maps(predict, target)
    results = _run(in_maps)
    return _postprocess(results, predict, target)


if __name__ == "__main__":
    rng = np.random.default_rng(0)
    predict = rng.standard_normal((B, N, 3)).astype(np.float32)
    target = rng.standard_normal((B, M, 3)).astype(np.float32)
    out = kernel(predict, target)
    exp_x = 0.0
    exp_y = 0.0
    for b in range(B):
        d = ((predict[b][:, None, :] - target[b][None, :, :]) ** 2).sum(-1)
        exp_x += d.min(axis=1).sum()
        exp_y += d.min(axis=0).sum()
    exp = exp_x / (B * N) + exp_y / (B * M)
    print("kernel:", out, "expected:", exp, "rel err:",
          abs(out - exp) / abs(exp))
